# revision 14
# baseline (speedup 1.0000x reference)
"""RBF-kernel attention on 8 TRN2 NeuronCores.

Math (per reference): scores = exp(-gamma*SCALE*dist), dist = ||qh_s - kh_t||^2,
kept only on the STRICT upper triangle (t > s), out = scores @ vh, then @ Wo.

Factorization: scores[s,t] = exp(2c*qk[s,t]) * exp(-c*kn[t]) * exp(-c*qn[s]),
c = gamma_h*SCALE. The 2c factor is folded into Wk HOST-SIDE (wk pre-scaled),
so the device computes qk~ = 2c*qk directly and the score exp is a PURE exp
(no scale/bias APs -> exp calls merge across the head pair into [128,1024]
tiles). exp(-c*kn[t]) folds into v (v' = B*v, per-partition scalars).
exp(-c*qn[s]) scales outT columns at PSUM->SBUF copy time (A broadcast built
with a tiny ones-matmul). The max(dist,0) clamp is a no-op off the diagonal
(dist ~ 50 >> rounding noise; the diagonal is masked out).

Sharding: core c = (batch b=c//4, head-group g=c%4); each core computes 4
heads of one batch end-to-end and a PARTIAL final output [S, E] (bf16)
through its Wo row slice. The host sums the 4 partials per batch. No
collectives.
"""
import sys
sys.path.insert(0, '/opt/trn_rl_repo')
import math
import numpy as np
import ml_dtypes

from concourse import bass, bacc, tile, mybir, bass_utils

F32 = mybir.dt.float32
BF16 = mybir.dt.bfloat16
AF = mybir.ActivationFunctionType
ALU = mybir.AluOpType

B, S, E, H = 2, 2048, 1024, 16
D = E // H              # 64
SCALE = 1.0 / math.sqrt(D)
N_CORES = 8
HPC = H // 4            # heads per core = 4
HD = HPC * D            # 256 local head dims
NKT = E // 128          # 8 contraction k-tiles for projections
NST = S // 128          # 16 t-tiles of 128
NSC = S // 512          # 4 s-chunks of 512

_nc_cache = {}


def build_graph():
    if 'nc' in _nc_cache:
        return _nc_cache['nc']
    nc = bacc.Bacc("TRN2", target_bir_lowering=False, debug=False,
                   num_devices=N_CORES)

    q_in = nc.dram_tensor("qb", [S, E], F32, kind="ExternalInput").ap()
    wq_in = nc.dram_tensor("wq", [E, HD], F32, kind="ExternalInput").ap()
    wk_in = nc.dram_tensor("wk", [E, HD], F32, kind="ExternalInput").ap()
    wv_in = nc.dram_tensor("wv", [E, HD], F32, kind="ExternalInput").ap()
    wo_in = nc.dram_tensor("wo", [HD, E], F32, kind="ExternalInput").ap()
    # negck[j, m] = -1/(4*SCALE*gamma_{2m+j})  (kn path; kh is 2c-scaled)
    negck_in = nc.dram_tensor("negck", [2, 2], F32, kind="ExternalInput").ap()
    # negcq[0, h] = -SCALE*gamma_h  (qn path)
    negcq_in = nc.dram_tensor("negcq", [1, HPC], F32, kind="ExternalInput").ap()
    out_d = nc.dram_tensor("out", [S, E], BF16, kind="ExternalOutput").ap()

    id_bf = nc.inline_tensor(np.eye(128, dtype=ml_dtypes.bfloat16), name="idbf")
    id_f32 = nc.inline_tensor(np.eye(4, dtype=np.float32), name="idf32")
    hsel_np = np.zeros((128, HPC), dtype=ml_dtypes.bfloat16)
    for j in range(HPC):
        hsel_np[64 * (j % 2):64 * (j % 2) + 64, j] = 1
    hsel_c = nc.inline_tensor(hsel_np, name="hsel")
    ones64_c = nc.inline_tensor(np.ones((1, 64), dtype=ml_dtypes.bfloat16),
                                name="ones64")

    with tile.TileContext(nc) as tc:
        with tc.tile_pool(name="persist", bufs=1) as P, \
             tc.tile_pool(name="wpool", bufs=1) as WP:
            qhT = [P.tile([128, S], BF16, name=f"qhT{m}", tag=f"qhT{m}")
                   for m in range(2)]
            khT = [P.tile([128, S], BF16, name=f"khT{m}", tag=f"khT{m}")
                   for m in range(2)]
            vp = [P.tile([128, HD], BF16, name=f"vp{w}", tag=f"vp{w}")
                  for w in range(NST)]
            outT = [P.tile([128, S], BF16, name=f"outT{m}", tag=f"outT{m}")
                    for m in range(2)]
            kn_m = [P.tile([2, S], F32, name=f"kn{m}", tag=f"kn{m}")
                    for m in range(2)]
            knT = P.tile([128, 4 * NST], F32, name="knT", tag="knT")
            BT = P.tile([128, 4 * NST], F32, name="BT", tag="BT")
            A_bc = [P.tile([64, 512], F32, name=f"Abc{h}{sj}", tag=f"Abc{h}{sj}")
                    for h in range(HPC) for sj in range(NSC)]
            id_t = P.tile([128, 128], BF16, name="id", tag="id")
            id4_t = P.tile([4, 4], F32, name="id4", tag="id4")
            hsel_t = P.tile([128, HPC], BF16, name="hsel", tag="hsel")
            ones64_t = P.tile([1, 64], BF16, name="ones64", tag="ones64")
            negck_t = P.tile([2, 2], F32, name="negck", tag="negck")
            negcq_t = P.tile([1, HPC], F32, name="negcq", tag="negcq")
            wqb = [WP.tile([128, HD], BF16, name=f"wqb{k}", tag=f"wqb{k}")
                   for k in range(NKT)]
            wkb = [WP.tile([128, HD], BF16, name=f"wkb{k}", tag=f"wkb{k}")
                   for k in range(NKT)]
            wvb = [WP.tile([128, HD], BF16, name=f"wvb{k}", tag=f"wvb{k}")
                   for k in range(NKT)]
            wob = [WP.tile([128, E], BF16, name=f"wob{k}", tag=f"wob{k}")
                   for k in range(2)]

            nc.sync.dma_start(id_t[:], id_bf.ap())
            nc.sync.dma_start(id4_t[:], id_f32.ap())
            nc.sync.dma_start(hsel_t[:], hsel_c.ap())
            nc.sync.dma_start(ones64_t[:], ones64_c.ap())
            nc.sync.dma_start(negck_t[:], negck_in)
            nc.sync.dma_start(negcq_t[:], negcq_in)

            # ---- load + cast weights (bf16) ----
            with tc.tile_pool(name="wtmp", bufs=3) as WT:
                for k in range(NKT):
                    for src, dst in ((wk_in, wkb), (wv_in, wvb), (wq_in, wqb)):
                        t = WT.tile([128, HD], F32, name="wtmp", tag="wtmp")
                        nc.sync.dma_start(t[:], src[128 * k:128 * k + 128, :])
                        nc.vector.tensor_copy(dst[k][:], t[:])
                for k in range(2):
                    t = WT.tile([128, E], F32, name="wotmp", tag="wotmp")
                    nc.sync.dma_start(t[:], wo_in[128 * k:128 * k + 128, :])
                    nc.vector.tensor_copy(wob[k][:], t[:])

            # ---- stage 1: transpose q -> qT (bf16) ----
            with tc.tile_pool(name="qT", bufs=1) as QTP:
                qT = [QTP.tile([128, S], BF16, name=f"qT{e}", tag=f"qT{e}")
                      for e in range(NKT)]
                with tc.tile_pool(name="qraw", bufs=3) as QR, \
                     tc.tile_pool(name="trps", bufs=3, space="PSUM") as TRP:
                    for si in range(NST):
                        qrb = QR.tile([128, E], BF16, name="qraw", tag="qraw")
                        nc.gpsimd.dma_start(qrb[:],
                                            q_in[128 * si:128 * si + 128, :])
                        for e in range(NKT):
                            tp = TRP.tile([128, 128], BF16, name="trp", tag="trp")
                            nc.tensor.transpose(
                                tp[:], qrb[:, 128 * e:128 * e + 128], id_t[:])
                            nc.vector.tensor_copy(
                                qT[e][:, 128 * si:128 * si + 128], tp[:])

                # ---- stage 2: projections (khT, kn, BT, vp, qhT, A) ----
                with tc.tile_pool(name="pjps", bufs=3, space="PSUM") as PJ, \
                     tc.tile_pool(name="sq", bufs=2) as SQ, \
                     tc.tile_pool(name="nps", bufs=2, space="PSUM") as NP:
                    for m in range(2):
                        for n in range(NSC):
                            ps = PJ.tile([128, 512], F32, name="pjk", tag="pj")
                            for k in range(NKT):
                                nc.tensor.matmul(
                                    ps[:], wkb[k][:, 128 * m:128 * m + 128],
                                    qT[k][:, 512 * n:512 * n + 512],
                                    start=(k == 0), stop=(k == NKT - 1))
                            nc.vector.tensor_copy(
                                khT[m][:, 512 * n:512 * n + 512], ps[:])

                    # kn (from 2c-scaled khT; negck corrects the 4c^2 factor)
                    for m in range(2):
                        sq = SQ.tile([128, S], BF16, name="sqk", tag="sqk")
                        nc.vector.tensor_tensor(sq[:], khT[m][:], khT[m][:],
                                                op=ALU.mult)
                        for n in range(NSC):
                            ps = NP.tile([2, 512], F32, name="np", tag="np")
                            nc.tensor.matmul(
                                ps[:], hsel_t[:, 2 * m:2 * m + 2],
                                sq[:, 512 * n:512 * n + 512],
                                start=True, stop=True)
                            nc.vector.tensor_scalar(
                                kn_m[m][0:2, 512 * n:512 * n + 512], ps[:],
                                negck_t[0:2, m:m + 1], None, op0=ALU.mult)
                    # knT via PE transpose; BT = exp(knT)
                    with tc.tile_pool(name="ktps", bufs=2, space="PSUM") as KT:
                        for ti in range(NST):
                            for m in range(2):
                                ps = KT.tile([128, 2], F32, name="kt", tag="kt")
                                nc.tensor.transpose(
                                    ps[:], kn_m[m][:, 128 * ti:128 * ti + 128],
                                    id4_t[0:2, 0:2])
                                nc.vector.tensor_copy(
                                    knT[:, 4 * ti + 2 * m:4 * ti + 2 * m + 2],
                                    ps[:])
                    nc.scalar.activation(BT[:], knT[:], AF.Exp)

                    # vh with fused B-scale: vp = vh * BT[t]
                    for w in range(NST):
                        ps = PJ.tile([128, HD], F32, name="pjv", tag="pj")
                        for k in range(NKT):
                            nc.tensor.matmul(
                                ps[:], qT[k][:, 128 * w:128 * w + 128],
                                wvb[k][:],
                                start=(k == 0), stop=(k == NKT - 1))
                        for h in range(HPC):
                            nc.vector.tensor_scalar(
                                vp[w][:, 64 * h:64 * h + 64],
                                ps[:, 64 * h:64 * h + 64],
                                BT[:, 4 * w + h:4 * w + h + 1], None,
                                op0=ALU.mult)

                    # qhT projections
                    for m in range(2):
                        for n in range(NSC):
                            ps = PJ.tile([128, 512], F32, name="pjk", tag="pj")
                            for k in range(NKT):
                                nc.tensor.matmul(
                                    ps[:], wqb[k][:, 128 * m:128 * m + 128],
                                    qT[k][:, 512 * n:512 * n + 512],
                                    start=(k == 0), stop=(k == NKT - 1))
                            nc.vector.tensor_copy(
                                qhT[m][:, 512 * n:512 * n + 512], ps[:])

                    # qn per head -> A = exp(-c*qn) (bf16), broadcast to
                    # [64, 512] tiles via ones-matmul
                    A_sb = [SQ.tile([1, S], BF16, name=f"Asb{h}",
                                    tag=f"Asb{h}", bufs=1)
                            for h in range(HPC)]
                    for m in range(2):
                        sq = SQ.tile([128, S], BF16, name="sqq", tag="sqq")
                        nc.vector.tensor_tensor(sq[:], qhT[m][:], qhT[m][:],
                                                op=ALU.mult)
                        for j in range(2):
                            h = 2 * m + j
                            qn = SQ.tile([1, S], F32, name="qnh", tag="qnh")
                            for n in range(NSC):
                                ps = NP.tile([1, 512], F32, name="np1",
                                             tag="np")
                                nc.tensor.matmul(
                                    ps[:], hsel_t[:, j:j + 1],
                                    sq[:, 512 * n:512 * n + 512],
                                    start=True, stop=True)
                                nc.vector.tensor_scalar(
                                    qn[0:1, 512 * n:512 * n + 512], ps[:],
                                    negcq_t[0:1, h:h + 1], None, op0=ALU.mult)
                            nc.scalar.activation(A_sb[h][:], qn[:], AF.Exp)
                    for h in range(HPC):
                        for sj in range(NSC):
                            ps = NP.tile([64, 512], F32, name="abps",
                                         tag="np")
                            nc.tensor.matmul(
                                ps[:], ones64_t[:],
                                A_sb[h][0:1, 512 * sj:512 * sj + 512],
                                start=True, stop=True)
                            nc.vector.tensor_copy(A_bc[h * NSC + sj][:], ps[:])

            # ---- stage 3: attention (strict upper triangle) ----
            with tc.tile_pool(name="qkps", bufs=3, space="PSUM") as QK, \
                 tc.tile_pool(name="otps", bufs=2, space="PSUM") as OT, \
                 tc.tile_pool(name="ep", bufs=4) as EP:
                for sj in range(NSC):
                    ot_ps = [OT.tile([128, 512], F32, name="ot", tag="ot")
                             for m in range(2)]
                    for m in range(2):
                        nc.vector.memset(ot_ps[m][:], 0.0)
                    for ti in range(4 * sj, NST):
                        last = (ti == NST - 1)
                        for m in range(2):
                            qk2 = QK.tile([128, 1024], F32, name="qk", tag="qk")
                            for hl in range(2):
                                base = 64 * hl
                                nc.tensor.matmul(
                                    qk2[:, 512 * hl:512 * hl + 512],
                                    khT[m][base:base + 64,
                                           128 * ti:128 * ti + 128],
                                    qhT[m][base:base + 64,
                                           512 * sj:512 * sj + 512],
                                    start=True, stop=True)
                            et2 = EP.tile([128, 1024], BF16, name="et", tag="et")
                            nc.scalar.activation(et2[:], qk2[:], AF.Exp)
                            if ti < 4 * (sj + 1):  # diagonal-crossing tile
                                nc.gpsimd.affine_select(
                                    et2[:], et2[:], pattern=[[0, 2], [-1, 512]],
                                    compare_op=ALU.is_gt, fill=0.0,
                                    base=128 * ti - 512 * sj,
                                    channel_multiplier=1)
                            for hl in range(2):
                                h = 2 * m + hl
                                base = 64 * hl
                                nc.tensor.matmul(
                                    ot_ps[m][base:base + 64, :],
                                    vp[ti][:, 64 * h:64 * h + 64],
                                    et2[:, 512 * hl:512 * hl + 512],
                                    start=False, stop=last,
                                    skip_group_check=True)
                    for m in range(2):
                        for hl in range(2):
                            h = 2 * m + hl
                            base = 64 * hl
                            nc.vector.tensor_tensor(
                                outT[m][base:base + 64, 512 * sj:512 * sj + 512],
                                ot_ps[m][base:base + 64, :],
                                A_bc[h * NSC + sj][:], op=ALU.mult)

            # ---- stage 4: final partial = outT.T @ Wo_slice (bf16 out) ----
            with tc.tile_pool(name="fps", bufs=4, space="PSUM") as FP, \
                 tc.tile_pool(name="fout", bufs=3) as FO:
                for w in range(NST):
                    fo = FO.tile([128, E], BF16, name="fo", tag="fo")
                    for n in range(2):
                        ps = FP.tile([128, 512], F32, name="fp", tag="fp")
                        for k in range(2):
                            nc.tensor.matmul(
                                ps[:], outT[k][:, 128 * w:128 * w + 128],
                                wob[k][:, 512 * n:512 * n + 512],
                                start=(k == 0), stop=(k == 1))
                        nc.vector.tensor_copy(fo[:, 512 * n:512 * n + 512],
                                              ps[:])
                    nc.sync.dma_start(out_d[128 * w:128 * w + 128, :], fo[:])

    nc.compile()
    _nc_cache['nc'] = nc
    return nc


def shard_inputs(q, Wq, Wk, Wv, Wo, gamma):
    in_maps = []
    for c in range(N_CORES):
        b, g = c // 4, c % 4
        cols = slice(HD * g, HD * (g + 1))
        gam = gamma[HPC * g:HPC * (g + 1)].astype(np.float64)
        c_h = gam * SCALE
        wk_scaled = Wk[:, cols].astype(np.float64).copy()
        for h in range(HPC):
            wk_scaled[:, 64 * h:64 * h + 64] *= 2.0 * c_h[h]
        negck = (-1.0 / (4.0 * c_h)).reshape(2, 2).T  # [j, m] = head 2m+j
        negcq = (-c_h).reshape(1, HPC)
        in_maps.append(dict(
            qb=np.ascontiguousarray(q[b]),
            wq=np.ascontiguousarray(Wq[:, cols]),
            wk=np.ascontiguousarray(wk_scaled.astype(np.float32)),
            wv=np.ascontiguousarray(Wv[:, cols]),
            wo=np.ascontiguousarray(Wo[cols, :]),
            negck=np.ascontiguousarray(negck.astype(np.float32)),
            negcq=np.ascontiguousarray(negcq.astype(np.float32)),
        ))
    return in_maps


def kernel(q, Wq, Wk, Wv, Wo, gamma):
    q = np.asarray(q, dtype=np.float32)
    Wq = np.asarray(Wq, dtype=np.float32)
    Wk = np.asarray(Wk, dtype=np.float32)
    Wv = np.asarray(Wv, dtype=np.float32)
    Wo = np.asarray(Wo, dtype=np.float32)
    gamma = np.asarray(gamma, dtype=np.float32)

    nc = build_graph()
    in_maps = shard_inputs(q, Wq, Wk, Wv, Wo, gamma)
    res = bass_utils.run_bass_kernel_spmd(nc, in_maps,
                                          core_ids=list(range(N_CORES)))
    out = np.zeros((B, S, E), dtype=np.float32)
    for c in range(N_CORES):
        out[c // 4] += np.asarray(res.results[c]["out"], dtype=np.float32)
    return out


# revision 15
# speedup vs baseline: 1.1130x; 1.1130x over previous
"""RBF-kernel attention on 8 TRN2 NeuronCores.

Math (per reference): scores = exp(-gamma*SCALE*dist), dist = ||qh_s - kh_t||^2,
kept only on the STRICT upper triangle (t > s), out = scores @ vh, then @ Wo.

Factorization: scores[s,t] = exp(2c*qk[s,t]) * exp(-c*kn[t]) * exp(-c*qn[s]),
c = gamma_h*SCALE. The 2c factor is folded into Wk HOST-SIDE, so the device
computes qk~ = 2c*qk and the score exp is a PURE exp (merged over the head
pair into [128,1024] tiles). exp(-c*kn[t]) folds into v (v' = B*v).
exp(-c*qn[s]) scales outT at PSUM->SBUF copy time. max(dist,0) is a no-op
off the masked diagonal. q arrives HOST-pre-transposed as qbT [E, S] f32.
Strict-upper masking: diagonal-crossing t-tiles use column-span shrink plus
an inline bf16 mask multiply on DVE.

Sharding: core c = (batch b=c//4, head-group g=c%4); each core computes 4
heads of one batch and a PARTIAL final output [S, E] (bf16) through its Wo
row slice; the host sums the 4 partials per batch. No collectives.
"""
import sys
sys.path.insert(0, '/opt/trn_rl_repo')
import math
import numpy as np
import ml_dtypes

from concourse import bass, bacc, tile, mybir, bass_utils

F32 = mybir.dt.float32
BF16 = mybir.dt.bfloat16
AF = mybir.ActivationFunctionType
ALU = mybir.AluOpType

B, S, E, H = 2, 2048, 1024, 16
D = E // H
SCALE = 1.0 / math.sqrt(D)
N_CORES = 8
HPC = H // 4
HD = HPC * D            # 256
NKT = E // 128          # 8
NST = S // 128          # 16
NSC = S // 512          # 4

_nc_cache = {}


def build_graph():
    if 'nc' in _nc_cache:
        return _nc_cache['nc']
    nc = bacc.Bacc("TRN2", target_bir_lowering=False, debug=False,
                   num_devices=N_CORES)

    qT_in = nc.dram_tensor("qbT", [E, S], F32, kind="ExternalInput").ap()
    wq_in = nc.dram_tensor("wq", [E, HD], F32, kind="ExternalInput").ap()
    wk_in = nc.dram_tensor("wk", [E, HD], F32, kind="ExternalInput").ap()
    wv_in = nc.dram_tensor("wv", [E, HD], F32, kind="ExternalInput").ap()
    wo_in = nc.dram_tensor("wo", [HD, E], F32, kind="ExternalInput").ap()
    negck_in = nc.dram_tensor("negck", [2, 2], F32, kind="ExternalInput").ap()
    negcq_in = nc.dram_tensor("negcq", [1, HPC], F32, kind="ExternalInput").ap()
    out_d = nc.dram_tensor("out", [S, E], BF16, kind="ExternalOutput").ap()

    id_f32 = nc.inline_tensor(np.eye(4, dtype=np.float32), name="idf32")
    hsel_np = np.zeros((128, HPC), dtype=ml_dtypes.bfloat16)
    for j in range(HPC):
        hsel_np[64 * (j % 2):64 * (j % 2) + 64, j] = 1
    hsel_c = nc.inline_tensor(hsel_np, name="hsel")
    ones64_c = nc.inline_tensor(np.ones((1, 64), dtype=ml_dtypes.bfloat16),
                                name="ones64")
    # masks[r][p, f] = 1 if p + 128r - f > 0 (strict upper within diag tile)
    mk = np.zeros((4, 128, 512), dtype=ml_dtypes.bfloat16)
    for r in range(4):
        pp = np.arange(128)[:, None]
        ff = np.arange(512)[None, :]
        mk[r] = (pp + 128 * r - ff > 0).astype(ml_dtypes.bfloat16)
    mask_c = [nc.inline_tensor(mk[r], name=f"mask{r}") for r in range(4)]

    with tile.TileContext(nc) as tc:
        with tc.tile_pool(name="persist", bufs=1) as P, \
             tc.tile_pool(name="wpool", bufs=1) as WP:
            qT = [P.tile([128, S], BF16, name=f"qT{e}", tag=f"qT{e}")
                  for e in range(NKT)]
            qhT = [P.tile([128, S], BF16, name=f"qhT{m}", tag=f"qhT{m}")
                   for m in range(2)]
            khT = [P.tile([128, S], BF16, name=f"khT{m}", tag=f"khT{m}")
                   for m in range(2)]
            vp = [P.tile([128, HD], BF16, name=f"vp{w}", tag=f"vp{w}")
                  for w in range(NST)]
            outT = [P.tile([128, S], BF16, name=f"outT{m}", tag=f"outT{m}")
                    for m in range(2)]
            kn_m = [P.tile([2, S], F32, name=f"kn{m}", tag=f"kn{m}")
                    for m in range(2)]
            knT = P.tile([128, 4 * NST], F32, name="knT", tag="knT")
            BT = P.tile([128, 4 * NST], F32, name="BT", tag="BT")
            A_sb = [P.tile([1, S], BF16, name=f"Asb{h}", tag=f"Asb{h}")
                    for h in range(HPC)]
            A_bc = [P.tile([64, 512], F32, name=f"Abc{h}{sj}", tag=f"Abc{h}{sj}")
                    for h in range(HPC) for sj in range(NSC)]
            id4_t = P.tile([4, 4], F32, name="id4", tag="id4")
            hsel_t = P.tile([128, HPC], BF16, name="hsel", tag="hsel")
            ones64_t = P.tile([1, 64], BF16, name="ones64", tag="ones64")
            negck_t = P.tile([2, 2], F32, name="negck", tag="negck")
            negcq_t = P.tile([1, HPC], F32, name="negcq", tag="negcq")
            mask_t = [P.tile([128, 512], BF16, name=f"mask{r}", tag=f"mask{r}")
                      for r in range(4)]
            wqb = [WP.tile([128, HD], BF16, name=f"wqb{k}", tag=f"wqb{k}")
                   for k in range(NKT)]
            wkb = [WP.tile([128, HD], BF16, name=f"wkb{k}", tag=f"wkb{k}")
                   for k in range(NKT)]
            wvb = [WP.tile([128, HD], BF16, name=f"wvb{k}", tag=f"wvb{k}")
                   for k in range(NKT)]
            wob = [WP.tile([128, E], BF16, name=f"wob{k}", tag=f"wob{k}")
                   for k in range(2)]

            nc.sync.dma_start(id4_t[:], id_f32.ap())
            nc.sync.dma_start(hsel_t[:], hsel_c.ap())
            nc.sync.dma_start(ones64_t[:], ones64_c.ap())
            nc.sync.dma_start(negck_t[:], negck_in)
            nc.sync.dma_start(negcq_t[:], negcq_in)
            for r in range(4):
                nc.sync.dma_start(mask_t[r][:], mask_c[r].ap())

            # ---- load + cast weights and qT (bf16) ----
            with tc.tile_pool(name="wtmp", bufs=4) as WT:
                for e in range(NKT):
                    t = WT.tile([128, S], F32, name="qtmp", tag="qtmp")
                    nc.sync.dma_start(t[:], qT_in[128 * e:128 * e + 128, :])
                    nc.vector.tensor_copy(qT[e][:], t[:])
                for k in range(NKT):
                    for src, dst in ((wk_in, wkb), (wv_in, wvb), (wq_in, wqb)):
                        t = WT.tile([128, HD], F32, name="wtmp", tag="wtmp")
                        nc.sync.dma_start(t[:], src[128 * k:128 * k + 128, :])
                        nc.vector.tensor_copy(dst[k][:], t[:])
                for k in range(2):
                    t = WT.tile([128, E], F32, name="wotmp", tag="wotmp")
                    nc.sync.dma_start(t[:], wo_in[128 * k:128 * k + 128, :])
                    nc.vector.tensor_copy(wob[k][:], t[:])

            # ---- shared PSUM pool for projections/norms/qk ----
            with tc.tile_pool(name="ps", bufs=3, space="PSUM") as PS, \
                 tc.tile_pool(name="otps", bufs=2, space="PSUM") as OT, \
                 tc.tile_pool(name="sq", bufs=2) as SQ, \
                 tc.tile_pool(name="ep", bufs=4) as EP:

                # khT projections
                for m in range(2):
                    for n in range(NSC):
                        ps = PS.tile([128, 1024], F32, name="ps", tag="ps")
                        for k in range(NKT):
                            nc.tensor.matmul(
                                ps[:, 0:512], wkb[k][:, 128 * m:128 * m + 128],
                                qT[k][:, 512 * n:512 * n + 512],
                                start=(k == 0), stop=(k == NKT - 1))
                        nc.vector.tensor_copy(
                            khT[m][:, 512 * n:512 * n + 512], ps[:, 0:512])
                # kn -> knT -> BT
                for m in range(2):
                    sq = SQ.tile([128, S], BF16, name="sqk", tag="sqk")
                    nc.vector.tensor_tensor(sq[:], khT[m][:], khT[m][:],
                                            op=ALU.mult)
                    for n in range(NSC):
                        ps = PS.tile([128, 1024], F32, name="psn", tag="ps")
                        nc.tensor.matmul(ps[0:2, 0:512],
                                         hsel_t[:, 2 * m:2 * m + 2],
                                         sq[:, 512 * n:512 * n + 512],
                                         start=True, stop=True)
                        nc.vector.tensor_scalar(
                            kn_m[m][0:2, 512 * n:512 * n + 512], ps[0:2, 0:512],
                            negck_t[0:2, m:m + 1], None, op0=ALU.mult)
                for ti in range(NST):
                    for m in range(2):
                        ps = PS.tile([128, 1024], F32, name="pst", tag="ps")
                        nc.tensor.transpose(
                            ps[0:128, 0:2], kn_m[m][:, 128 * ti:128 * ti + 128],
                            id4_t[0:2, 0:2])
                        nc.vector.tensor_copy(
                            knT[:, 4 * ti + 2 * m:4 * ti + 2 * m + 2],
                            ps[0:128, 0:2])
                nc.scalar.activation(BT[:], knT[:], AF.Exp)

                # vp = vh * BT[t]
                for w in range(NST):
                    ps = PS.tile([128, 1024], F32, name="psv", tag="ps")
                    for k in range(NKT):
                        nc.tensor.matmul(
                            ps[:, 0:HD], qT[k][:, 128 * w:128 * w + 128],
                            wvb[k][:], start=(k == 0), stop=(k == NKT - 1))
                    for h in range(HPC):
                        nc.vector.tensor_scalar(
                            vp[w][:, 64 * h:64 * h + 64],
                            ps[:, 64 * h:64 * h + 64],
                            BT[:, 4 * w + h:4 * w + h + 1], None, op0=ALU.mult)

                # interleaved: per s-chunk, project qhT + A, then attention
                for sj in range(NSC):
                    # qhT chunk sj
                    for m in range(2):
                        ps = PS.tile([128, 1024], F32, name="psq", tag="ps")
                        for k in range(NKT):
                            nc.tensor.matmul(
                                ps[:, 0:512], wqb[k][:, 128 * m:128 * m + 128],
                                qT[k][:, 512 * sj:512 * sj + 512],
                                start=(k == 0), stop=(k == NKT - 1))
                        nc.vector.tensor_copy(
                            qhT[m][:, 512 * sj:512 * sj + 512], ps[:, 0:512])
                    # qn chunk + A chunk + A_bc
                    for m in range(2):
                        sq = SQ.tile([128, 512], BF16, name="sqq", tag="sqq")
                        nc.vector.tensor_tensor(
                            sq[:], qhT[m][:, 512 * sj:512 * sj + 512],
                            qhT[m][:, 512 * sj:512 * sj + 512], op=ALU.mult)
                        for j in range(2):
                            h = 2 * m + j
                            ps = PS.tile([128, 1024], F32, name="psa", tag="ps")
                            nc.tensor.matmul(ps[0:1, 0:512],
                                             hsel_t[:, j:j + 1], sq[:],
                                             start=True, stop=True)
                            qn = SQ.tile([1, 512], F32, name="qnh", tag="qnh")
                            nc.vector.tensor_scalar(
                                qn[:], ps[0:1, 0:512],
                                negcq_t[0:1, h:h + 1], None, op0=ALU.mult)
                            nc.scalar.activation(
                                A_sb[h][0:1, 512 * sj:512 * sj + 512], qn[:],
                                AF.Exp)
                    for h in range(HPC):
                        ps = PS.tile([128, 1024], F32, name="psb", tag="ps")
                        nc.tensor.matmul(
                            ps[0:64, 0:512], ones64_t[:],
                            A_sb[h][0:1, 512 * sj:512 * sj + 512],
                            start=True, stop=True)
                        nc.vector.tensor_copy(A_bc[h * NSC + sj][:],
                                              ps[0:64, 0:512])

                    # attention for this s-chunk
                    ot_ps = [OT.tile([128, 512], F32, name="ot", tag="ot")
                             for m in range(2)]
                    for m in range(2):
                        nc.vector.memset(ot_ps[m][:], 0.0)
                    for ti in range(4 * sj, NST):
                        r = ti - 4 * sj
                        span = min(512, 128 * (r + 1))
                        diag = r < 4
                        last = (ti == NST - 1)
                        for m in range(2):
                            qk2 = PS.tile([128, 1024], F32, name="qk", tag="ps")
                            for hl in range(2):
                                base = 64 * hl
                                nc.tensor.matmul(
                                    qk2[:, 512 * hl:512 * hl + span],
                                    khT[m][base:base + 64,
                                           128 * ti:128 * ti + 128],
                                    qhT[m][base:base + 64,
                                           512 * sj:512 * sj + span],
                                    start=True, stop=True)
                            et2 = EP.tile([128, 1024], BF16, name="et",
                                          tag="et")
                            if span == 512:
                                nc.scalar.activation(et2[:], qk2[:], AF.Exp)
                            else:
                                for hl in range(2):
                                    nc.scalar.activation(
                                        et2[:, 512 * hl:512 * hl + span],
                                        qk2[:, 512 * hl:512 * hl + span],
                                        AF.Exp)
                            if diag:
                                for hl in range(2):
                                    nc.vector.tensor_tensor(
                                        et2[:, 512 * hl:512 * hl + span],
                                        et2[:, 512 * hl:512 * hl + span],
                                        mask_t[r][:, 0:span], op=ALU.mult)
                            for hl in range(2):
                                h = 2 * m + hl
                                base = 64 * hl
                                nc.tensor.matmul(
                                    ot_ps[m][base:base + 64, 0:span],
                                    vp[ti][:, 64 * h:64 * h + 64],
                                    et2[:, 512 * hl:512 * hl + span],
                                    start=False, stop=last,
                                    skip_group_check=True)
                    for m in range(2):
                        for hl in range(2):
                            h = 2 * m + hl
                            base = 64 * hl
                            nc.vector.tensor_tensor(
                                outT[m][base:base + 64,
                                        512 * sj:512 * sj + 512],
                                ot_ps[m][base:base + 64, :],
                                A_bc[h * NSC + sj][:], op=ALU.mult)

            # ---- final partial = outT.T @ Wo_slice (bf16 out) ----
            with tc.tile_pool(name="fps", bufs=4, space="PSUM") as FP, \
                 tc.tile_pool(name="fout", bufs=3) as FO:
                for w in range(NST):
                    fo = FO.tile([128, E], BF16, name="fo", tag="fo")
                    for n in range(2):
                        ps = FP.tile([128, 512], F32, name="fp", tag="fp")
                        for k in range(2):
                            nc.tensor.matmul(
                                ps[:], outT[k][:, 128 * w:128 * w + 128],
                                wob[k][:, 512 * n:512 * n + 512],
                                start=(k == 0), stop=(k == 1))
                        nc.vector.tensor_copy(fo[:, 512 * n:512 * n + 512],
                                              ps[:])
                    nc.sync.dma_start(out_d[128 * w:128 * w + 128, :], fo[:])

    nc.compile()
    _nc_cache['nc'] = nc
    return nc


def shard_inputs(q, Wq, Wk, Wv, Wo, gamma):
    in_maps = []
    for c in range(N_CORES):
        b, g = c // 4, c % 4
        cols = slice(HD * g, HD * (g + 1))
        gam = gamma[HPC * g:HPC * (g + 1)].astype(np.float64)
        c_h = gam * SCALE
        wk_scaled = Wk[:, cols].astype(np.float64).copy()
        for h in range(HPC):
            wk_scaled[:, 64 * h:64 * h + 64] *= 2.0 * c_h[h]
        negck = (-1.0 / (4.0 * c_h)).reshape(2, 2).T
        negcq = (-c_h).reshape(1, HPC)
        in_maps.append(dict(
            qbT=np.ascontiguousarray(q[b].T),
            wq=np.ascontiguousarray(Wq[:, cols]),
            wk=np.ascontiguousarray(wk_scaled.astype(np.float32)),
            wv=np.ascontiguousarray(Wv[:, cols]),
            wo=np.ascontiguousarray(Wo[cols, :]),
            negck=np.ascontiguousarray(negck.astype(np.float32)),
            negcq=np.ascontiguousarray(negcq.astype(np.float32)),
        ))
    return in_maps


def kernel(q, Wq, Wk, Wv, Wo, gamma):
    q = np.asarray(q, dtype=np.float32)
    Wq = np.asarray(Wq, dtype=np.float32)
    Wk = np.asarray(Wk, dtype=np.float32)
    Wv = np.asarray(Wv, dtype=np.float32)
    Wo = np.asarray(Wo, dtype=np.float32)
    gamma = np.asarray(gamma, dtype=np.float32)

    nc = build_graph()
    in_maps = shard_inputs(q, Wq, Wk, Wv, Wo, gamma)
    res = bass_utils.run_bass_kernel_spmd(nc, in_maps,
                                          core_ids=list(range(N_CORES)))
    out = np.zeros((B, S, E), dtype=np.float32)
    for c in range(N_CORES):
        out[c // 4] += np.asarray(res.results[c]["out"], dtype=np.float32)
    return out


# revision 16
# speedup vs baseline: 1.1439x; 1.0278x over previous
"""RBF-kernel attention on 8 TRN2 NeuronCores.

Math (per reference): scores = exp(-gamma*SCALE*dist), dist = ||qh_s - kh_t||^2,
kept only on the STRICT upper triangle (t > s), out = scores @ vh, then @ Wo.

Factorization: scores[s,t] = exp(2c*qk[s,t]) * exp(-c*kn[t]) * exp(-c*qn[s]),
c = gamma_h*SCALE. The 2c factor is folded into Wk HOST-SIDE, so the device
computes qk~ = 2c*qk and the score exp is a PURE exp (merged over the head
pair into [128,1024] tiles). exp(-c*kn[t]) folds into v (v' = B*v).
exp(-c*qn[s]) scales outT at PSUM->SBUF copy time. max(dist,0) is a no-op
off the masked diagonal. q arrives HOST-pre-transposed as qbT [E, S] f32.
Strict-upper masking: diagonal-crossing t-tiles use column-span shrink plus
an inline bf16 mask multiply on DVE.

Sharding: core c = (batch b=c//4, head-group g=c%4); each core computes 4
heads of one batch and a PARTIAL final output [S, E] (bf16) through its Wo
row slice; the host sums the 4 partials per batch. No collectives.
"""
import sys
sys.path.insert(0, '/opt/trn_rl_repo')
import math
import numpy as np
import ml_dtypes

from concourse import bass, bacc, tile, mybir, bass_utils

F32 = mybir.dt.float32
BF16 = mybir.dt.bfloat16
AF = mybir.ActivationFunctionType
ALU = mybir.AluOpType

B, S, E, H = 2, 2048, 1024, 16
D = E // H
SCALE = 1.0 / math.sqrt(D)
N_CORES = 8
HPC = H // 4
HD = HPC * D            # 256
NKT = E // 128          # 8
NST = S // 128          # 16
NSC = S // 512          # 4

_nc_cache = {}


def build_graph():
    if 'nc' in _nc_cache:
        return _nc_cache['nc']
    nc = bacc.Bacc("TRN2", target_bir_lowering=False, debug=False,
                   num_devices=N_CORES)

    qT_in = nc.dram_tensor("qbT", [E, S], BF16, kind="ExternalInput").ap()
    wq_in = nc.dram_tensor("wq", [E, HD], BF16, kind="ExternalInput").ap()
    wk_in = nc.dram_tensor("wk", [E, HD], BF16, kind="ExternalInput").ap()
    wv_in = nc.dram_tensor("wv", [E, HD], BF16, kind="ExternalInput").ap()
    wo_in = nc.dram_tensor("wo", [HD, E], BF16, kind="ExternalInput").ap()
    negck_in = nc.dram_tensor("negck", [2, 2], F32, kind="ExternalInput").ap()
    negcq_in = nc.dram_tensor("negcq", [1, HPC], F32, kind="ExternalInput").ap()
    out_d = nc.dram_tensor("out", [S, E], BF16, kind="ExternalOutput").ap()

    id_f32 = nc.inline_tensor(np.eye(4, dtype=np.float32), name="idf32")
    hsel_np = np.zeros((128, HPC), dtype=ml_dtypes.bfloat16)
    for j in range(HPC):
        hsel_np[64 * (j % 2):64 * (j % 2) + 64, j] = 1
    hsel_c = nc.inline_tensor(hsel_np, name="hsel")
    ones64_c = nc.inline_tensor(np.ones((1, 64), dtype=ml_dtypes.bfloat16),
                                name="ones64")
    # masks[r][p, f] = 1 if p + 128r - f > 0 (strict upper within diag tile)
    mk = np.zeros((4, 128, 512), dtype=ml_dtypes.bfloat16)
    for r in range(4):
        pp = np.arange(128)[:, None]
        ff = np.arange(512)[None, :]
        mk[r] = (pp + 128 * r - ff > 0).astype(ml_dtypes.bfloat16)
    mask_c = [nc.inline_tensor(mk[r], name=f"mask{r}") for r in range(4)]

    with tile.TileContext(nc) as tc:
        with tc.tile_pool(name="persist", bufs=1) as P, \
             tc.tile_pool(name="wpool", bufs=1) as WP:
            qT = [P.tile([128, S], BF16, name=f"qT{e}", tag=f"qT{e}")
                  for e in range(NKT)]
            qhT = [P.tile([128, S], BF16, name=f"qhT{m}", tag=f"qhT{m}")
                   for m in range(2)]
            khT = [P.tile([128, S], BF16, name=f"khT{m}", tag=f"khT{m}")
                   for m in range(2)]
            vp = [P.tile([128, HD], BF16, name=f"vp{w}", tag=f"vp{w}")
                  for w in range(NST)]
            outT = [P.tile([128, S], BF16, name=f"outT{m}", tag=f"outT{m}")
                    for m in range(2)]
            kn_m = [P.tile([2, S], F32, name=f"kn{m}", tag=f"kn{m}")
                    for m in range(2)]
            knT = P.tile([128, 4 * NST], F32, name="knT", tag="knT")
            BT = P.tile([128, 4 * NST], F32, name="BT", tag="BT")
            A_sb = [P.tile([1, S], BF16, name=f"Asb{h}", tag=f"Asb{h}")
                    for h in range(HPC)]
            A_bc = [P.tile([64, 512], F32, name=f"Abc{h}{sj}", tag=f"Abc{h}{sj}")
                    for h in range(HPC) for sj in range(NSC)]
            id4_t = P.tile([4, 4], F32, name="id4", tag="id4")
            hsel_t = P.tile([128, HPC], BF16, name="hsel", tag="hsel")
            ones64_t = P.tile([1, 64], BF16, name="ones64", tag="ones64")
            negck_t = P.tile([2, 2], F32, name="negck", tag="negck")
            negcq_t = P.tile([1, HPC], F32, name="negcq", tag="negcq")
            mask_t = [P.tile([128, 512], BF16, name=f"mask{r}", tag=f"mask{r}")
                      for r in range(4)]
            wqb = [WP.tile([128, HD], BF16, name=f"wqb{k}", tag=f"wqb{k}")
                   for k in range(NKT)]
            wkb = [WP.tile([128, HD], BF16, name=f"wkb{k}", tag=f"wkb{k}")
                   for k in range(NKT)]
            wvb = [WP.tile([128, HD], BF16, name=f"wvb{k}", tag=f"wvb{k}")
                   for k in range(NKT)]
            wob = [WP.tile([128, E], BF16, name=f"wob{k}", tag=f"wob{k}")
                   for k in range(2)]

            nc.sync.dma_start(id4_t[:], id_f32.ap())
            nc.sync.dma_start(hsel_t[:], hsel_c.ap())
            nc.sync.dma_start(ones64_t[:], ones64_c.ap())
            nc.sync.dma_start(negck_t[:], negck_in)
            nc.sync.dma_start(negcq_t[:], negcq_in)
            for r in range(4):
                nc.sync.dma_start(mask_t[r][:], mask_c[r].ap())

            # ---- load bf16 weights and qT directly ----
            for e in range(NKT):
                nc.sync.dma_start(qT[e][:], qT_in[128 * e:128 * e + 128, :])
            for k in range(NKT):
                for srct, dst in ((wk_in, wkb), (wv_in, wvb), (wq_in, wqb)):
                    nc.sync.dma_start(dst[k][:], srct[128 * k:128 * k + 128, :])
            for k in range(2):
                nc.sync.dma_start(wob[k][:], wo_in[128 * k:128 * k + 128, :])

            # ---- shared PSUM pool for projections/norms/qk ----
            with tc.tile_pool(name="ps", bufs=3, space="PSUM") as PS, \
                 tc.tile_pool(name="otps", bufs=2, space="PSUM") as OT, \
                 tc.tile_pool(name="sq", bufs=2) as SQ, \
                 tc.tile_pool(name="ep", bufs=4) as EP:

                # khT projections
                for m in range(2):
                    for n in range(NSC):
                        ps = PS.tile([128, 1024], F32, name="ps", tag="ps")
                        for k in range(NKT):
                            nc.tensor.matmul(
                                ps[:, 0:512], wkb[k][:, 128 * m:128 * m + 128],
                                qT[k][:, 512 * n:512 * n + 512],
                                start=(k == 0), stop=(k == NKT - 1))
                        nc.vector.tensor_copy(
                            khT[m][:, 512 * n:512 * n + 512], ps[:, 0:512])
                # kn -> knT -> BT
                for m in range(2):
                    sq = SQ.tile([128, S], BF16, name="sqk", tag="sqk")
                    nc.vector.tensor_tensor(sq[:], khT[m][:], khT[m][:],
                                            op=ALU.mult)
                    for n in range(NSC):
                        ps = PS.tile([128, 1024], F32, name="psn", tag="ps")
                        nc.tensor.matmul(ps[0:2, 0:512],
                                         hsel_t[:, 2 * m:2 * m + 2],
                                         sq[:, 512 * n:512 * n + 512],
                                         start=True, stop=True)
                        nc.vector.tensor_scalar(
                            kn_m[m][0:2, 512 * n:512 * n + 512], ps[0:2, 0:512],
                            negck_t[0:2, m:m + 1], None, op0=ALU.mult)
                for ti in range(NST):
                    for m in range(2):
                        ps = PS.tile([128, 1024], F32, name="pst", tag="ps")
                        nc.tensor.transpose(
                            ps[0:128, 0:2], kn_m[m][:, 128 * ti:128 * ti + 128],
                            id4_t[0:2, 0:2])
                        nc.vector.tensor_copy(
                            knT[:, 4 * ti + 2 * m:4 * ti + 2 * m + 2],
                            ps[0:128, 0:2])
                nc.scalar.activation(BT[:], knT[:], AF.Exp)

                # vp = vh * BT[t]
                for w in range(NST):
                    ps = PS.tile([128, 1024], F32, name="psv", tag="ps")
                    for k in range(NKT):
                        nc.tensor.matmul(
                            ps[:, 0:HD], qT[k][:, 128 * w:128 * w + 128],
                            wvb[k][:], start=(k == 0), stop=(k == NKT - 1))
                    for h in range(HPC):
                        nc.vector.tensor_scalar(
                            vp[w][:, 64 * h:64 * h + 64],
                            ps[:, 64 * h:64 * h + 64],
                            BT[:, 4 * w + h:4 * w + h + 1], None, op0=ALU.mult)

                # interleaved: per s-chunk, project qhT + A, then attention
                for sj in range(NSC):
                    # qhT chunk sj
                    for m in range(2):
                        ps = PS.tile([128, 1024], F32, name="psq", tag="ps")
                        for k in range(NKT):
                            nc.tensor.matmul(
                                ps[:, 0:512], wqb[k][:, 128 * m:128 * m + 128],
                                qT[k][:, 512 * sj:512 * sj + 512],
                                start=(k == 0), stop=(k == NKT - 1))
                        nc.vector.tensor_copy(
                            qhT[m][:, 512 * sj:512 * sj + 512], ps[:, 0:512])
                    # qn chunk + A chunk + A_bc
                    for m in range(2):
                        sq = SQ.tile([128, 512], BF16, name="sqq", tag="sqq")
                        nc.vector.tensor_tensor(
                            sq[:], qhT[m][:, 512 * sj:512 * sj + 512],
                            qhT[m][:, 512 * sj:512 * sj + 512], op=ALU.mult)
                        for j in range(2):
                            h = 2 * m + j
                            ps = PS.tile([128, 1024], F32, name="psa", tag="ps")
                            nc.tensor.matmul(ps[0:1, 0:512],
                                             hsel_t[:, j:j + 1], sq[:],
                                             start=True, stop=True)
                            qn = SQ.tile([1, 512], F32, name="qnh", tag="qnh")
                            nc.vector.tensor_scalar(
                                qn[:], ps[0:1, 0:512],
                                negcq_t[0:1, h:h + 1], None, op0=ALU.mult)
                            nc.scalar.activation(
                                A_sb[h][0:1, 512 * sj:512 * sj + 512], qn[:],
                                AF.Exp)
                    for h in range(HPC):
                        ps = PS.tile([128, 1024], F32, name="psb", tag="ps")
                        nc.tensor.matmul(
                            ps[0:64, 0:512], ones64_t[:],
                            A_sb[h][0:1, 512 * sj:512 * sj + 512],
                            start=True, stop=True)
                        nc.vector.tensor_copy(A_bc[h * NSC + sj][:],
                                              ps[0:64, 0:512])

                    # attention for this s-chunk
                    ot_ps = [OT.tile([128, 512], F32, name="ot", tag="ot")
                             for m in range(2)]
                    for m in range(2):
                        nc.vector.memset(ot_ps[m][:], 0.0)
                    for ti in range(4 * sj, NST):
                        r = ti - 4 * sj
                        span = min(512, 128 * (r + 1))
                        diag = r < 4
                        last = (ti == NST - 1)
                        for m in range(2):
                            qk2 = PS.tile([128, 1024], F32, name="qk", tag="ps")
                            for hl in range(2):
                                base = 64 * hl
                                nc.tensor.matmul(
                                    qk2[:, 512 * hl:512 * hl + span],
                                    khT[m][base:base + 64,
                                           128 * ti:128 * ti + 128],
                                    qhT[m][base:base + 64,
                                           512 * sj:512 * sj + span],
                                    start=True, stop=True)
                            et2 = EP.tile([128, 1024], BF16, name="et",
                                          tag="et")
                            if span == 512:
                                nc.scalar.activation(et2[:], qk2[:], AF.Exp)
                            else:
                                for hl in range(2):
                                    nc.scalar.activation(
                                        et2[:, 512 * hl:512 * hl + span],
                                        qk2[:, 512 * hl:512 * hl + span],
                                        AF.Exp)
                            if diag:
                                for hl in range(2):
                                    nc.vector.tensor_tensor(
                                        et2[:, 512 * hl:512 * hl + span],
                                        et2[:, 512 * hl:512 * hl + span],
                                        mask_t[r][:, 0:span], op=ALU.mult)
                            for hl in range(2):
                                h = 2 * m + hl
                                base = 64 * hl
                                nc.tensor.matmul(
                                    ot_ps[m][base:base + 64, 0:span],
                                    vp[ti][:, 64 * h:64 * h + 64],
                                    et2[:, 512 * hl:512 * hl + span],
                                    start=False, stop=last,
                                    skip_group_check=True)
                    for m in range(2):
                        for hl in range(2):
                            h = 2 * m + hl
                            base = 64 * hl
                            nc.vector.tensor_tensor(
                                outT[m][base:base + 64,
                                        512 * sj:512 * sj + 512],
                                ot_ps[m][base:base + 64, :],
                                A_bc[h * NSC + sj][:], op=ALU.mult)

                    # final partial for this chunk's s-windows (bf16 out)
                    for w in range(4 * sj, 4 * sj + 4):
                        fo = EP.tile([128, E], BF16, name="fo", tag="fo")
                        for n in range(2):
                            ps = PS.tile([128, 1024], F32, name="fp", tag="ps")
                            for k in range(2):
                                nc.tensor.matmul(
                                    ps[:, 0:512],
                                    outT[k][:, 128 * w:128 * w + 128],
                                    wob[k][:, 512 * n:512 * n + 512],
                                    start=(k == 0), stop=(k == 1))
                            nc.vector.tensor_copy(
                                fo[:, 512 * n:512 * n + 512], ps[:, 0:512])
                        nc.sync.dma_start(out_d[128 * w:128 * w + 128, :],
                                          fo[:])

    nc.compile()
    _nc_cache['nc'] = nc
    return nc


def shard_inputs(q, Wq, Wk, Wv, Wo, gamma):
    in_maps = []
    for c in range(N_CORES):
        b, g = c // 4, c % 4
        cols = slice(HD * g, HD * (g + 1))
        gam = gamma[HPC * g:HPC * (g + 1)].astype(np.float64)
        c_h = gam * SCALE
        wk_scaled = Wk[:, cols].astype(np.float64).copy()
        for h in range(HPC):
            wk_scaled[:, 64 * h:64 * h + 64] *= 2.0 * c_h[h]
        negck = (-1.0 / (4.0 * c_h)).reshape(2, 2).T
        negcq = (-c_h).reshape(1, HPC)
        in_maps.append(dict(
            qbT=np.ascontiguousarray(q[b].T.astype(ml_dtypes.bfloat16)),
            wq=np.ascontiguousarray(Wq[:, cols].astype(ml_dtypes.bfloat16)),
            wk=np.ascontiguousarray(wk_scaled.astype(np.float32).astype(ml_dtypes.bfloat16)),
            wv=np.ascontiguousarray(Wv[:, cols].astype(ml_dtypes.bfloat16)),
            wo=np.ascontiguousarray(Wo[cols, :].astype(ml_dtypes.bfloat16)),
            negck=np.ascontiguousarray(negck.astype(np.float32)),
            negcq=np.ascontiguousarray(negcq.astype(np.float32)),
        ))
    return in_maps


def kernel(q, Wq, Wk, Wv, Wo, gamma):
    q = np.asarray(q, dtype=np.float32)
    Wq = np.asarray(Wq, dtype=np.float32)
    Wk = np.asarray(Wk, dtype=np.float32)
    Wv = np.asarray(Wv, dtype=np.float32)
    Wo = np.asarray(Wo, dtype=np.float32)
    gamma = np.asarray(gamma, dtype=np.float32)

    nc = build_graph()
    in_maps = shard_inputs(q, Wq, Wk, Wv, Wo, gamma)
    res = bass_utils.run_bass_kernel_spmd(nc, in_maps,
                                          core_ids=list(range(N_CORES)))
    out = np.zeros((B, S, E), dtype=np.float32)
    for c in range(N_CORES):
        out[c // 4] += np.asarray(res.results[c]["out"], dtype=np.float32)
    return out
